# revision 82
# baseline (speedup 1.0000x reference)
"""Trainium2 Bass kernel for NeuralCausalModel (per-variable 3-layer MLP).

Math: wx = x @ A.T; per variable i:
    h1 = relu(cat([x, wx[:,i]]) @ W1[i].T + b1[i])
    h2 = relu(h1 @ W2[i].T + b2[i]);  out[:,i] = h2 @ W3[i] + b3[i]
The concat column is folded into W1 host-side (W1eff = W1[:,:, :V] +
W1[:,:,V:]*A), removing the adjacency matmul and the ragged K=257.

Sharding: V=256 split across 8 cores (32 vars/core), x replicated.

All matmul operands are fp16: same PE rate as f32r (measured), half the
weight DMA traffic, 2x DVE rate on fp16 SBUF tensors, and rel err
~5e-4, far inside the 2e-2 gate.

Layer 3 (out[:,i] = W3[i]@h2 + b3) runs entirely OFF the PE: the
e-axis of W2/b2/W3 is permuted host-side so h2's four 128-row tiles
are sign-pure in W3 (tile0 +, tile1 mixed, tiles 2/3 -), |W3| rides
the W2 weights so layer 2's activation is a plain bias+relu, the
mixed tile applies its per-partition +-1 vector in one DVE STT op
followed by two DVE subtracts, and the partition sum + sigma*x + b3
run as a gpsimd partition_all_reduce + tensor_scalar on the otherwise
idle Pool engine. All input-dependence (permutation, signs, sigma)
lives in tensor data, never in access patterns, so one SPMD program
serves all 8 cores.

Measured constraints that shaped this design: with 8 cores running
concurrently the sustained matmul rate is ~250-260ns per N=512 matmul
instruction (instruction-stream contention; single-core microbenches
run the same stream at ~210ns), so PE instruction COUNT is the global
binder. Moving layer 3's 64 matmuls + 32 ACT ops per core to Pool cut
the kernel from ~435us to ~414-422us; the remaining 1536 matmuls are
the mathematical minimum at the N=512 PSUM-bank limit. Stationary
pair-reuse matmul order, narrow [128,512] PSUM tiles with an 8-bank
rotation, and in-order per-variable emission all measured fastest;
software pipelining, 2-bank-wide PSUM consumers, and merged weight
DMAs each measured neutral-to-worse.
"""

import contextlib

import numpy as np

V, D, B = 256, 512, 1024
NCORES = 8
VL = V // NCORES  # 32 variables per core

import os as _os

MM_DTYPE = _os.environ.get("KERNEL_MM_DTYPE", "f16")

_CACHE = {}


def _np_mm_dtype():
    if MM_DTYPE == "bf16":
        import ml_dtypes

        return ml_dtypes.bfloat16
    if MM_DTYPE == "f16":
        return np.float16
    return np.float32


def _build(reps=1):
    key = (MM_DTYPE, reps)
    if key in _CACHE:
        return _CACHE[key]

    import sys

    if "/opt/trn_rl_repo" not in sys.path:
        sys.path.insert(0, "/opt/trn_rl_repo")

    import concourse.mybir as mybir
    import concourse.tile as tile
    from concourse import bacc, bass_isa

    f32 = mybir.dt.float32
    mdt = {
        "f32r": mybir.dt.float32r,
        "bf16": mybir.dt.bfloat16,
        "f16": mybir.dt.float16,
        "f32": mybir.dt.float32,
    }[MM_DTYPE]

    nc = bacc.Bacc("TRN2", target_bir_lowering=False, debug=False)

    xT = nc.declare_dram_parameter("xT", [V, B], mdt, isOutput=False)
    w1 = nc.declare_dram_parameter("w1t", [VL, V, D], mdt, isOutput=False)
    w2 = nc.declare_dram_parameter("w2t", [VL, D, D], mdt, isOutput=False)
    # per-variable +-1 sigma polarity, applied in the final Pool op
    sgv = nc.declare_dram_parameter("sgv", [1, VL], f32, isOutput=False)
    b1 = nc.declare_dram_parameter("b1t", [128, 128], f32, isOutput=False)
    # 4 bias columns per variable (tiles t0+, t1 mixed, t2-, t3-);
    # |W3| is folded into w2t host-side
    biA = nc.declare_dram_parameter("biA", [128, VL * 4], f32, isOutput=False)
    # per-partition +-1 signs for the mixed tile t1 (sigma-space)
    sg3 = nc.declare_dram_parameter("sg3", [128, VL], f32, isOutput=False)
    b3 = nc.declare_dram_parameter("b3t", [1, VL], f32, isOutput=False)
    out = nc.declare_dram_parameter("out", [VL, B], f32, isOutput=True)

    Relu = mybir.ActivationFunctionType.Relu
    Ident = mybir.ActivationFunctionType.Identity
    add = mybir.AluOpType.add
    sub = mybir.AluOpType.subtract
    amax = mybir.AluOpType.max
    mult = mybir.AluOpType.mult

    with tile.TileContext(nc) as tc:
        with (
            tc.tile_pool(name="const", bufs=1) as const_pool,
            tc.tile_pool(name="w1p", bufs=6) as w1_pool,
            tc.tile_pool(name="w2p", bufs=12) as w2_pool,
            tc.tile_pool(name="h1p", bufs=8) as h1_pool,
            tc.tile_pool(name="tp", bufs=10) as t_pool,
            tc.tile_pool(name="accp", bufs=3) as acc_pool,
            tc.tile_pool(name="redp", bufs=2) as red_pool,
            tc.tile_pool(name="m3p", bufs=4) as m3_pool,
            tc.tile_pool(name="psp", bufs=8, space="PSUM") as ps_pool,
        ):
            # x tiles issue from the ACT-engine DMA queue so the SP engine
            # can issue variable-0's weight DMAs immediately; split so the
            # first L1 matmul (cols 0:512 of both halves) can start early
            xt0 = const_pool.tile([128, B], mdt, tag="xt0")
            xt1 = const_pool.tile([128, B], mdt, tag="xt1")
            nc.scalar.dma_start(xt0[:, 0:512], xT[0:128, 0:512])
            nc.scalar.dma_start(xt1[:, 0:512], xT[128:256, 0:512])
            nc.scalar.dma_start(xt0[:, 512:B], xT[0:128, 512:B])
            nc.scalar.dma_start(xt1[:, 512:B], xT[128:256, 512:B])
            b1sb = const_pool.tile([128, 128], f32, tag="b1sb")
            nc.gpsimd.dma_start(b1sb[:], b1[:])
            # less-urgent constants go via the idle Pool engine's DMA queue
            biAsb = const_pool.tile([128, VL * 4], f32, tag="biAsb")
            nc.gpsimd.dma_start(biAsb[:], biA[:])
            sg3sb = const_pool.tile([128, VL], f32, tag="sg3sb")
            nc.gpsimd.dma_start(sg3sb[:], sg3[:])
            sgvsb = const_pool.tile([1, VL], f32, tag="sgvsb")
            nc.gpsimd.dma_start(sgvsb[:], sgv[:])
            b3sb = const_pool.tile([1, VL], f32, tag="b3sb")
            nc.gpsimd.dma_start(b3sb[:], b3[:])

            def emit_l3(v, acc):
                # partition sum on the otherwise-idle Pool engine (frees the
                # PE of 2 matmuls and the ACT of its bias op per variable;
                # PE instruction issue rate is the global binder), then
                # sigma*x + b3 also on Pool, then DMA out
                red = red_pool.tile([128, B], f32, tag="red", name="red")
                m3sb = m3_pool.tile([1, B], f32, tag="m3sb", name="m3sb")
                nc.gpsimd.partition_all_reduce(
                    red[:], acc[:], 128, bass_isa.ReduceOp.add
                )
                nc.gpsimd.tensor_scalar(
                    m3sb[0:1, :],
                    red[0:1, :],
                    sgvsb[0:1, v : v + 1],
                    b3sb[0:1, v : v + 1],
                    op0=mult,
                    op1=add,
                )
                nc.sync.dma_start(out[v : v + 1, :], m3sb[:])

            rep_ctx = tc.For_i(0, reps, 1) if reps > 1 else contextlib.nullcontext()
            with rep_ctx:
                pending = []
                for v in range(VL):
                    w1t = [
                        w1_pool.tile([128, D], mdt, tag="w1t", name=f"w1t_{k}")
                        for k in range(2)
                    ]
                    for kk in range(2):
                        nc.sync.dma_start(
                            w1t[kk][:], w1[v, kk * 128 : (kk + 1) * 128, :]
                        )
                    w2t = [
                        w2_pool.tile([128, D], mdt, tag="w2t", name=f"w2t_{k}")
                        for k in range(4)
                    ]
                    for dd in range(4):
                        nc.sync.dma_start(
                            w2t[dd][:], w2[v, dd * 128 : (dd + 1) * 128, :]
                        )

                    # L1: stationary w1t[kk][:,ms] loaded once, both batch
                    # halves run against it before switching; bias+relu on DVE
                    h1t = [
                        h1_pool.tile([128, B], mdt, tag="h1t", name=f"h1t_{k}")
                        for k in range(4)
                    ]
                    for dd in range(4):
                        ms = slice(dd * 128, (dd + 1) * 128)
                        ps = [
                            ps_pool.tile([128, 512], f32, tag="ps", name="ps")
                            for _ in range(2)
                        ]
                        for kk, xt in ((0, xt0), (1, xt1)):
                            for bb in range(2):
                                bs = slice(bb * 512, (bb + 1) * 512)
                                nc.tensor.matmul(
                                    ps[bb][:],
                                    w1t[kk][:, ms],
                                    xt[:, bs],
                                    start=(kk == 0),
                                    stop=(kk == 1),
                                )
                        # drain the two PSUM halves on DVE and ACT
                        # concurrently: h1t[dd] completes ~2x sooner, so
                        # L2's first accumulation chain (which needs
                        # h1t[3]) never stalls the PE
                        nc.vector.tensor_scalar(
                            h1t[dd][:, 0:512],
                            ps[0][:],
                            b1sb[:, v * 4 + dd : v * 4 + dd + 1],
                            0.0,
                            op0=add,
                            op1=amax,
                        )
                        nc.scalar.activation(
                            h1t[dd][:, 512:B],
                            ps[1][:],
                            Relu,
                            bias=b1sb[:, v * 4 + dd : v * 4 + dd + 1],
                        )

                    # emit the previous variable's L3 here: its tree finishes
                    # while this variable's L1 matmuls run, so the in-order
                    # PE never waits on it
                    if pending:
                        emit_l3(*pending.pop())

                    # L2 + bias+relu on ACT (|W3| rides w2t)
                    tt = [
                        t_pool.tile([128, B], mdt, tag="tt", name=f"tt_{k}")
                        for k in range(4)
                    ]
                    for ee in range(4):
                        ms = slice(ee * 128, (ee + 1) * 128)
                        col = slice(v * 4 + ee, v * 4 + ee + 1)
                        ps2 = [
                            ps_pool.tile([128, 512], f32, tag="ps", name="ps")
                            for _ in range(2)
                        ]
                        for dd in range(4):
                            for bb in range(2):
                                bs = slice(bb * 512, (bb + 1) * 512)
                                nc.tensor.matmul(
                                    ps2[bb][:],
                                    w2t[dd][:, ms],
                                    h1t[dd][:, bs],
                                    start=(dd == 0),
                                    stop=(dd == 3),
                                )
                        for bb in range(2):
                            bs = slice(bb * 512, (bb + 1) * 512)
                            nc.scalar.activation(
                                tt[ee][:, bs],
                                ps2[bb][:],
                                Relu,
                                bias=biAsb[:, col],
                            )

                    # signed tile tree on DVE: the mixed tile 1 applies its
                    # per-partition +-1 vector in one early STT op (t0/t1
                    # are ready first), then two sign-pure subtracts
                    acc = acc_pool.tile([128, B], mdt, tag="acc", name="acc")
                    nc.vector.scalar_tensor_tensor(
                        acc[:],
                        tt[1][:],
                        sg3sb[:, v : v + 1],
                        tt[0][:],
                        op0=mult,
                        op1=add,
                    )
                    nc.vector.tensor_tensor(acc[:], acc[:], tt[2][:], sub)
                    nc.vector.tensor_tensor(acc[:], acc[:], tt[3][:], sub)

                    pending.append((v, acc))
                emit_l3(*pending.pop())

    nc.compile()
    _CACHE[key] = nc
    return nc


def _prep_inputs(x, adjacency, W1, b1, W2, b2, W3, b3):
    mmnp = _np_mm_dtype()
    x = np.asarray(x, np.float32)
    A = np.asarray(adjacency, np.float32)
    W1 = np.asarray(W1, np.float32)
    W2 = np.asarray(W2, np.float32)
    W3 = np.asarray(W3, np.float32)
    b1 = np.asarray(b1, np.float32)
    b2 = np.asarray(b2, np.float32)
    b3 = np.asarray(b3, np.float32)

    W1eff = W1[:, :, :V] + W1[:, :, V : V + 1] * A[:, None, :]
    W1effT = np.ascontiguousarray(W1eff.transpose(0, 2, 1)).astype(mmnp)
    xT = np.ascontiguousarray(x.T).astype(mmnp)

    in_maps = []
    for c in range(NCORES):
        s = slice(c * VL, (c + 1) * VL)
        b1t = np.ascontiguousarray(
            b1[s].reshape(VL, 4, 128).transpose(2, 0, 1).reshape(128, VL * 4)
        )
        w2tc = np.empty((VL, D, D), mmnp)
        biA = np.empty((128, VL * 4), np.float32)
        sg3 = np.empty((128, VL), np.float32)
        onesc = np.empty((1, VL), np.float32)
        for j, gv in enumerate(range(c * VL, (c + 1) * VL)):
            w3v = W3[gv]
            pos = np.flatnonzero(w3v >= 0)
            neg = np.flatnonzero(w3v < 0)
            if len(pos) < 256:
                sigma, plus, minus = 1.0, pos, neg
            else:
                sigma, plus, minus = -1.0, neg, pos
            L = len(plus)
            assert 128 <= L <= 256, f"degenerate sign split L={L}"
            r = L - 128
            # tile0 pure plus; tile1 mixed (plus rows < r, minus rows >= r);
            # tiles 2/3 pure minus
            perm = np.concatenate(
                [
                    plus[0:128],
                    plus[128:L],
                    minus[0 : 128 - r],
                    minus[128 - r : 384 - r],
                ]
            )
            assert len(perm) == D
            aw3 = np.abs(w3v[perm])
            # |W3| rides the matmul weights: scale W2's (permuted) e-rows
            w2tc[j] = np.ascontiguousarray(
                (W2[gv][perm, :] * aw3[:, None]).T
            ).astype(mmnp)
            ab2 = aw3 * b2[gv][perm]
            for ee in range(4):
                biA[:, j * 4 + ee] = ab2[ee * 128 : (ee + 1) * 128]
            sg3[:r, j] = 1.0
            sg3[r:, j] = -1.0
            onesc[0, j] = sigma
        in_maps.append(
            {
                "xT": xT,
                "w1t": np.ascontiguousarray(W1effT[s]),
                "w2t": w2tc,
                "biA": biA,
                "sg3": sg3,
                "b1t": b1t,
                "b3t": np.ascontiguousarray(b3[s].reshape(1, VL)),
                "sgv": np.ascontiguousarray(onesc),
            }
        )
    return in_maps


def kernel(x, adjacency, W1, b1, W2, b2, W3, b3, _trace=False):
    import sys

    if "/opt/trn_rl_repo" not in sys.path:
        sys.path.insert(0, "/opt/trn_rl_repo")
    from concourse.bass_utils import run_bass_kernel_spmd

    nc = _build()
    in_maps = _prep_inputs(x, adjacency, W1, b1, W2, b2, W3, b3)
    res = run_bass_kernel_spmd(
        nc, in_maps, core_ids=list(range(NCORES)), trace=_trace
    )
    kernel.last_results = res
    outT = np.concatenate([res.results[c]["out"] for c in range(NCORES)], axis=0)
    return np.ascontiguousarray(outT.T.astype(np.float32))


kernel.last_results = None


# revision 85
# speedup vs baseline: 1.4341x; 1.4341x over previous
"""Trainium2 Bass kernel for NeuralCausalModel (per-variable 3-layer MLP).

Math: wx = x @ A.T; per variable i:
    h1 = relu(cat([x, wx[:,i]]) @ W1[i].T + b1[i])
    h2 = relu(h1 @ W2[i].T + b2[i]);  out[:,i] = h2 @ W3[i] + b3[i]
The concat column is folded into W1 host-side (W1eff = W1[:,:, :V] +
W1[:,:,V:]*A), removing the adjacency matmul and the ragged K=257.

Sharding: V=256 split across 8 cores (32 vars/core), x replicated.

All matmul operands are fp16: same PE rate as f32r (measured), half the
weight DMA traffic, 2x DVE rate on fp16 SBUF tensors, and rel err
~5e-4, far inside the 2e-2 gate.

Layer 3 (out[:,i] = W3[i]@h2 + b3) runs entirely OFF the PE: the
e-axis of W2/b2/W3 is permuted host-side so h2's four 128-row tiles
are sign-pure in W3 (tile0 +, tile1 mixed, tiles 2/3 -), |W3| rides
the W2 weights so layer 2's activation is a plain bias+relu, the
mixed tile applies its per-partition +-1 vector in one DVE STT op
followed by two DVE subtracts, and the partition sum + sigma*x + b3
run as a gpsimd partition_all_reduce + tensor_scalar on the otherwise
idle Pool engine. All input-dependence (permutation, signs, sigma)
lives in tensor data, never in access patterns, so one SPMD program
serves all 8 cores.

Measured constraints that shaped this design: with 8 cores running
concurrently the sustained matmul rate is ~250-260ns per N=512 matmul
instruction (instruction-stream contention; single-core microbenches
run the same stream at ~210ns), so PE instruction COUNT is the global
binder. Moving layer 3's 64 matmuls + 32 ACT ops per core to Pool cut
the kernel from ~435us to ~414-422us; the remaining 1536 matmuls are
the mathematical minimum at the N=512 PSUM-bank limit. Stationary
pair-reuse matmul order, narrow [128,512] PSUM tiles with an 8-bank
rotation, and in-order per-variable emission all measured fastest;
software pipelining, 2-bank-wide PSUM consumers, and merged weight
DMAs each measured neutral-to-worse.
"""

import contextlib

import numpy as np

V, D, B = 256, 512, 1024
NCORES = 8
VL = V // NCORES  # 32 variables per core

import os as _os

MM_DTYPE = _os.environ.get("KERNEL_MM_DTYPE", "f16")

_CACHE = {}


def _np_mm_dtype():
    if MM_DTYPE == "bf16":
        import ml_dtypes

        return ml_dtypes.bfloat16
    if MM_DTYPE == "f16":
        return np.float16
    return np.float32


def _build(reps=1):
    key = (MM_DTYPE, reps)
    if key in _CACHE:
        return _CACHE[key]

    import sys

    if "/opt/trn_rl_repo" not in sys.path:
        sys.path.insert(0, "/opt/trn_rl_repo")

    import concourse.mybir as mybir
    import concourse.tile as tile
    from concourse import bacc, bass_isa

    f32 = mybir.dt.float32
    mdt = {
        "f32r": mybir.dt.float32r,
        "bf16": mybir.dt.bfloat16,
        "f16": mybir.dt.float16,
        "f32": mybir.dt.float32,
    }[MM_DTYPE]

    nc = bacc.Bacc("TRN2", target_bir_lowering=False, debug=False)

    xT = nc.declare_dram_parameter("xT", [V, B], mdt, isOutput=False)
    w1 = nc.declare_dram_parameter("w1t", [VL, V, D], mdt, isOutput=False)
    w2 = nc.declare_dram_parameter("w2t", [VL, D, D], mdt, isOutput=False)
    # per-variable +-1 sigma polarity, applied in the final Pool op
    sgv = nc.declare_dram_parameter("sgv", [1, VL], f32, isOutput=False)
    b1 = nc.declare_dram_parameter("b1t", [128, 128], f32, isOutput=False)
    # 4 bias columns per variable (tiles t0+, t1 mixed, t2-, t3-);
    # |W3| is folded into w2t host-side
    biA = nc.declare_dram_parameter("biA", [128, VL * 4], f32, isOutput=False)
    # per-partition +-1 signs for the mixed tile t1 (sigma-space)
    sg3 = nc.declare_dram_parameter("sg3", [128, VL], f32, isOutput=False)
    b3 = nc.declare_dram_parameter("b3t", [1, VL], f32, isOutput=False)
    out = nc.declare_dram_parameter("out", [VL, B], f32, isOutput=True)

    Relu = mybir.ActivationFunctionType.Relu
    Ident = mybir.ActivationFunctionType.Identity
    add = mybir.AluOpType.add
    sub = mybir.AluOpType.subtract
    amax = mybir.AluOpType.max
    mult = mybir.AluOpType.mult

    with tile.TileContext(nc) as tc:
        with (
            tc.tile_pool(name="const", bufs=1) as const_pool,
            tc.tile_pool(name="w1p", bufs=6) as w1_pool,
            tc.tile_pool(name="w2p", bufs=12) as w2_pool,
            tc.tile_pool(name="h1p", bufs=8) as h1_pool,
            tc.tile_pool(name="tp", bufs=10) as t_pool,
            tc.tile_pool(name="accp", bufs=3) as acc_pool,
            tc.tile_pool(name="redp", bufs=2) as red_pool,
            tc.tile_pool(name="m3p", bufs=4) as m3_pool,
            tc.tile_pool(name="psp", bufs=8, space="PSUM") as ps_pool,
        ):
            # x tiles issue from the ACT-engine DMA queue so the SP engine
            # can issue variable-0's weight DMAs immediately; split so the
            # first L1 matmul (cols 0:512 of both halves) can start early
            xt0 = const_pool.tile([128, B], mdt, tag="xt0")
            xt1 = const_pool.tile([128, B], mdt, tag="xt1")
            nc.scalar.dma_start(xt0[:, 0:512], xT[0:128, 0:512])
            nc.scalar.dma_start(xt1[:, 0:512], xT[128:256, 0:512])
            nc.scalar.dma_start(xt0[:, 512:B], xT[0:128, 512:B])
            nc.scalar.dma_start(xt1[:, 512:B], xT[128:256, 512:B])
            b1sb = const_pool.tile([128, 128], f32, tag="b1sb")
            nc.gpsimd.dma_start(b1sb[:], b1[:])
            # less-urgent constants go via the idle Pool engine's DMA queue
            biAsb = const_pool.tile([128, VL * 4], f32, tag="biAsb")
            nc.gpsimd.dma_start(biAsb[:], biA[:])
            sg3sb = const_pool.tile([128, VL], f32, tag="sg3sb")
            nc.gpsimd.dma_start(sg3sb[:], sg3[:])
            sgvsb = const_pool.tile([1, VL], f32, tag="sgvsb")
            nc.gpsimd.dma_start(sgvsb[:], sgv[:])
            b3sb = const_pool.tile([1, VL], f32, tag="b3sb")
            nc.gpsimd.dma_start(b3sb[:], b3[:])

            def emit_l3(v, acc):
                # partition sum on the otherwise-idle Pool engine (frees the
                # PE of 2 matmuls and the ACT of its bias op per variable;
                # PE instruction issue rate is the global binder), then
                # sigma*x + b3 also on Pool, then DMA out
                red = red_pool.tile([128, B], f32, tag="red", name="red")
                m3sb = m3_pool.tile([1, B], f32, tag="m3sb", name="m3sb")
                nc.gpsimd.partition_all_reduce(
                    red[:], acc[:], 128, bass_isa.ReduceOp.add
                )
                nc.gpsimd.tensor_scalar(
                    m3sb[0:1, :],
                    red[0:1, :],
                    sgvsb[0:1, v : v + 1],
                    b3sb[0:1, v : v + 1],
                    op0=mult,
                    op1=add,
                )
                nc.sync.dma_start(out[v : v + 1, :], m3sb[:])

            rep_ctx = tc.For_i(0, reps, 1) if reps > 1 else contextlib.nullcontext()
            with rep_ctx:
                pending = []
                for v in range(VL):
                    w1t = [
                        w1_pool.tile([128, D], mdt, tag="w1t", name=f"w1t_{k}")
                        for k in range(2)
                    ]
                    for kk in range(2):
                        nc.sync.dma_start(
                            w1t[kk][:], w1[v, kk * 128 : (kk + 1) * 128, :]
                        )
                    w2t = [
                        w2_pool.tile([128, D], mdt, tag="w2t", name=f"w2t_{k}")
                        for k in range(4)
                    ]
                    for dd in range(4):
                        nc.sync.dma_start(
                            w2t[dd][:], w2[v, dd * 128 : (dd + 1) * 128, :]
                        )

                    # L1: stationary w1t[kk][:,ms] loaded once, both batch
                    # halves run against it before switching; bias+relu on DVE
                    h1t = [
                        h1_pool.tile([128, B], mdt, tag="h1t", name=f"h1t_{k}")
                        for k in range(4)
                    ]
                    for dd in range(4):
                        ms = slice(dd * 128, (dd + 1) * 128)
                        ps = [
                            ps_pool.tile([128, 512], f32, tag="ps", name="ps")
                            for _ in range(2)
                        ]
                        for kk, xt in ((0, xt0), (1, xt1)):
                            for bb in range(2):
                                bs = slice(bb * 512, (bb + 1) * 512)
                                nc.tensor.matmul(
                                    ps[bb][:],
                                    w1t[kk][:, ms],
                                    xt[:, bs],
                                    start=(kk == 0),
                                    stop=(kk == 1),
                                )
                        # drain the two PSUM halves on DVE and ACT
                        # concurrently: h1t[dd] completes ~2x sooner, so
                        # L2's first accumulation chain (which needs
                        # h1t[3]) never stalls the PE
                        nc.vector.tensor_scalar(
                            h1t[dd][:, 0:512],
                            ps[0][:],
                            b1sb[:, v * 4 + dd : v * 4 + dd + 1],
                            0.0,
                            op0=add,
                            op1=amax,
                        )
                        nc.scalar.activation(
                            h1t[dd][:, 512:B],
                            ps[1][:],
                            Relu,
                            bias=b1sb[:, v * 4 + dd : v * 4 + dd + 1],
                        )

                    # emit the previous variable's L3 here: its tree finishes
                    # while this variable's L1 matmuls run, so the in-order
                    # PE never waits on it
                    if pending:
                        emit_l3(*pending.pop())

                    # L2 + bias+relu on ACT (|W3| rides w2t)
                    tt = [
                        t_pool.tile([128, B], mdt, tag="tt", name=f"tt_{k}")
                        for k in range(4)
                    ]
                    for ee in range(4):
                        ms = slice(ee * 128, (ee + 1) * 128)
                        col = slice(v * 4 + ee, v * 4 + ee + 1)
                        ps2 = [
                            ps_pool.tile([128, 512], f32, tag="ps", name="ps")
                            for _ in range(2)
                        ]
                        for dd in range(4):
                            for bb in range(2):
                                bs = slice(bb * 512, (bb + 1) * 512)
                                nc.tensor.matmul(
                                    ps2[bb][:],
                                    w2t[dd][:, ms],
                                    h1t[dd][:, bs],
                                    start=(dd == 0),
                                    stop=(dd == 3),
                                )
                        # drain L2's two PSUM halves on ACT and DVE
                        # concurrently (mirrors the L1 split): banks
                        # recycle ~2x faster so the matmul rotation
                        # never waits on a single consumer engine
                        nc.scalar.activation(
                            tt[ee][:, 0:512],
                            ps2[0][:],
                            Relu,
                            bias=biAsb[:, col],
                        )
                        nc.vector.tensor_scalar(
                            tt[ee][:, 512:B],
                            ps2[1][:],
                            biAsb[:, col],
                            0.0,
                            op0=add,
                            op1=amax,
                        )

                    # signed tile tree on DVE: the mixed tile 1 applies its
                    # per-partition +-1 vector in one early STT op (t0/t1
                    # are ready first), then two sign-pure subtracts
                    acc = acc_pool.tile([128, B], mdt, tag="acc", name="acc")
                    nc.vector.scalar_tensor_tensor(
                        acc[:],
                        tt[1][:],
                        sg3sb[:, v : v + 1],
                        tt[0][:],
                        op0=mult,
                        op1=add,
                    )
                    nc.vector.tensor_tensor(acc[:], acc[:], tt[2][:], sub)
                    nc.vector.tensor_tensor(acc[:], acc[:], tt[3][:], sub)

                    pending.append((v, acc))
                emit_l3(*pending.pop())

    nc.compile()
    _CACHE[key] = nc
    return nc


def _prep_inputs(x, adjacency, W1, b1, W2, b2, W3, b3):
    mmnp = _np_mm_dtype()
    x = np.asarray(x, np.float32)
    A = np.asarray(adjacency, np.float32)
    W1 = np.asarray(W1, np.float32)
    W2 = np.asarray(W2, np.float32)
    W3 = np.asarray(W3, np.float32)
    b1 = np.asarray(b1, np.float32)
    b2 = np.asarray(b2, np.float32)
    b3 = np.asarray(b3, np.float32)

    W1eff = W1[:, :, :V] + W1[:, :, V : V + 1] * A[:, None, :]
    W1effT = np.ascontiguousarray(W1eff.transpose(0, 2, 1)).astype(mmnp)
    xT = np.ascontiguousarray(x.T).astype(mmnp)

    in_maps = []
    for c in range(NCORES):
        s = slice(c * VL, (c + 1) * VL)
        b1t = np.ascontiguousarray(
            b1[s].reshape(VL, 4, 128).transpose(2, 0, 1).reshape(128, VL * 4)
        )
        w2tc = np.empty((VL, D, D), mmnp)
        biA = np.empty((128, VL * 4), np.float32)
        sg3 = np.empty((128, VL), np.float32)
        onesc = np.empty((1, VL), np.float32)
        for j, gv in enumerate(range(c * VL, (c + 1) * VL)):
            w3v = W3[gv]
            pos = np.flatnonzero(w3v >= 0)
            neg = np.flatnonzero(w3v < 0)
            if len(pos) < 256:
                sigma, plus, minus = 1.0, pos, neg
            else:
                sigma, plus, minus = -1.0, neg, pos
            L = len(plus)
            assert 128 <= L <= 256, f"degenerate sign split L={L}"
            r = L - 128
            # tile0 pure plus; tile1 mixed (plus rows < r, minus rows >= r);
            # tiles 2/3 pure minus
            perm = np.concatenate(
                [
                    plus[0:128],
                    plus[128:L],
                    minus[0 : 128 - r],
                    minus[128 - r : 384 - r],
                ]
            )
            assert len(perm) == D
            aw3 = np.abs(w3v[perm])
            # |W3| rides the matmul weights: scale W2's (permuted) e-rows
            w2tc[j] = np.ascontiguousarray(
                (W2[gv][perm, :] * aw3[:, None]).T
            ).astype(mmnp)
            ab2 = aw3 * b2[gv][perm]
            for ee in range(4):
                biA[:, j * 4 + ee] = ab2[ee * 128 : (ee + 1) * 128]
            sg3[:r, j] = 1.0
            sg3[r:, j] = -1.0
            onesc[0, j] = sigma
        in_maps.append(
            {
                "xT": xT,
                "w1t": np.ascontiguousarray(W1effT[s]),
                "w2t": w2tc,
                "biA": biA,
                "sg3": sg3,
                "b1t": b1t,
                "b3t": np.ascontiguousarray(b3[s].reshape(1, VL)),
                "sgv": np.ascontiguousarray(onesc),
            }
        )
    return in_maps


def kernel(x, adjacency, W1, b1, W2, b2, W3, b3, _trace=False):
    import sys

    if "/opt/trn_rl_repo" not in sys.path:
        sys.path.insert(0, "/opt/trn_rl_repo")
    from concourse.bass_utils import run_bass_kernel_spmd

    nc = _build()
    in_maps = _prep_inputs(x, adjacency, W1, b1, W2, b2, W3, b3)
    res = run_bass_kernel_spmd(
        nc, in_maps, core_ids=list(range(NCORES)), trace=_trace
    )
    kernel.last_results = res
    outT = np.concatenate([res.results[c]["out"] for c in range(NCORES)], axis=0)
    return np.ascontiguousarray(outT.T.astype(np.float32))


kernel.last_results = None


# revision 86
# speedup vs baseline: 1.4493x; 1.0106x over previous
"""Trainium2 Bass kernel for NeuralCausalModel (per-variable 3-layer MLP).

Math: wx = x @ A.T; per variable i:
    h1 = relu(cat([x, wx[:,i]]) @ W1[i].T + b1[i])
    h2 = relu(h1 @ W2[i].T + b2[i]);  out[:,i] = h2 @ W3[i] + b3[i]
The concat column is folded into W1 host-side (W1eff = W1[:,:, :V] +
W1[:,:,V:]*A), removing the adjacency matmul and the ragged K=257.

Sharding: V=256 split across 8 cores (32 vars/core), x replicated.

All matmul operands are fp16: same PE rate as f32r (measured), half the
weight DMA traffic, 2x DVE rate on fp16 SBUF tensors, and rel err
~5e-4, far inside the 2e-2 gate.

Layer 3 (out[:,i] = W3[i]@h2 + b3) runs entirely OFF the PE: the
e-axis of W2/b2/W3 is permuted host-side so h2's four 128-row tiles
are sign-pure in W3 (tile0 +, tile1 mixed, tiles 2/3 -), |W3| rides
the W2 weights so layer 2's activation is a plain bias+relu, the
mixed tile applies its per-partition +-1 vector in one DVE STT op
followed by two DVE subtracts, and the partition sum + sigma*x + b3
run as a gpsimd partition_all_reduce + tensor_scalar on the otherwise
idle Pool engine. All input-dependence (permutation, signs, sigma)
lives in tensor data, never in access patterns, so one SPMD program
serves all 8 cores.

Measured constraints that shaped this design: with 8 cores running
concurrently the sustained matmul rate is ~250-260ns per N=512 matmul
instruction (instruction-stream contention; single-core microbenches
run the same stream at ~210ns), so PE instruction COUNT is the global
binder. Moving layer 3's 64 matmuls + 32 ACT ops per core to Pool cut
the kernel from ~435us to ~414-422us; the remaining 1536 matmuls are
the mathematical minimum at the N=512 PSUM-bank limit. Stationary
pair-reuse matmul order, narrow [128,512] PSUM tiles with an 8-bank
rotation, and in-order per-variable emission all measured fastest;
software pipelining, 2-bank-wide PSUM consumers, and merged weight
DMAs each measured neutral-to-worse.
"""

import contextlib

import numpy as np

V, D, B = 256, 512, 1024
NCORES = 8
VL = V // NCORES  # 32 variables per core

import os as _os

MM_DTYPE = _os.environ.get("KERNEL_MM_DTYPE", "f16")

_CACHE = {}


def _np_mm_dtype():
    if MM_DTYPE == "bf16":
        import ml_dtypes

        return ml_dtypes.bfloat16
    if MM_DTYPE == "f16":
        return np.float16
    return np.float32


def _build(reps=1):
    key = (MM_DTYPE, reps)
    if key in _CACHE:
        return _CACHE[key]

    import sys

    if "/opt/trn_rl_repo" not in sys.path:
        sys.path.insert(0, "/opt/trn_rl_repo")

    import concourse.mybir as mybir
    import concourse.tile as tile
    from concourse import bacc, bass_isa

    f32 = mybir.dt.float32
    mdt = {
        "f32r": mybir.dt.float32r,
        "bf16": mybir.dt.bfloat16,
        "f16": mybir.dt.float16,
        "f32": mybir.dt.float32,
    }[MM_DTYPE]

    nc = bacc.Bacc("TRN2", target_bir_lowering=False, debug=False)

    xT = nc.declare_dram_parameter("xT", [V, B], mdt, isOutput=False)
    w1 = nc.declare_dram_parameter("w1t", [VL, V, D], mdt, isOutput=False)
    w2 = nc.declare_dram_parameter("w2t", [VL, D, D], mdt, isOutput=False)
    # per-variable +-1 sigma polarity, applied in the final Pool op
    sgv = nc.declare_dram_parameter("sgv", [1, VL], f32, isOutput=False)
    b1 = nc.declare_dram_parameter("b1t", [128, 128], f32, isOutput=False)
    # 4 bias columns per variable (tiles t0+, t1 mixed, t2-, t3-);
    # |W3| is folded into w2t host-side
    biA = nc.declare_dram_parameter("biA", [128, VL * 4], f32, isOutput=False)
    # per-partition +-1 signs for the mixed tile t1 (sigma-space)
    sg3 = nc.declare_dram_parameter("sg3", [128, VL], f32, isOutput=False)
    b3 = nc.declare_dram_parameter("b3t", [1, VL], f32, isOutput=False)
    out = nc.declare_dram_parameter("out", [VL, B], f32, isOutput=True)

    Relu = mybir.ActivationFunctionType.Relu
    Ident = mybir.ActivationFunctionType.Identity
    add = mybir.AluOpType.add
    sub = mybir.AluOpType.subtract
    amax = mybir.AluOpType.max
    mult = mybir.AluOpType.mult

    with tile.TileContext(nc) as tc:
        with (
            tc.tile_pool(name="const", bufs=1) as const_pool,
            tc.tile_pool(name="w1p", bufs=6) as w1_pool,
            tc.tile_pool(name="w2p", bufs=12) as w2_pool,
            tc.tile_pool(name="h1p", bufs=8) as h1_pool,
            tc.tile_pool(name="tp", bufs=10) as t_pool,
            tc.tile_pool(name="accp", bufs=3) as acc_pool,
            tc.tile_pool(name="redp", bufs=2) as red_pool,
            tc.tile_pool(name="m3p", bufs=4) as m3_pool,
            tc.tile_pool(name="psp", bufs=8, space="PSUM") as ps_pool,
        ):
            # x tiles issue from the ACT-engine DMA queue so the SP engine
            # can issue variable-0's weight DMAs immediately; split so the
            # first L1 matmul (cols 0:512 of both halves) can start early
            xt0 = const_pool.tile([128, B], mdt, tag="xt0")
            xt1 = const_pool.tile([128, B], mdt, tag="xt1")
            nc.scalar.dma_start(xt0[:, 0:512], xT[0:128, 0:512])
            nc.scalar.dma_start(xt1[:, 0:512], xT[128:256, 0:512])
            nc.scalar.dma_start(xt0[:, 512:B], xT[0:128, 512:B])
            nc.scalar.dma_start(xt1[:, 512:B], xT[128:256, 512:B])
            b1sb = const_pool.tile([128, 128], f32, tag="b1sb")
            nc.gpsimd.dma_start(b1sb[:], b1[:])
            # less-urgent constants go via the idle Pool engine's DMA queue
            biAsb = const_pool.tile([128, VL * 4], f32, tag="biAsb")
            nc.gpsimd.dma_start(biAsb[:], biA[:])
            sg3sb = const_pool.tile([128, VL], f32, tag="sg3sb")
            nc.gpsimd.dma_start(sg3sb[:], sg3[:])
            sgvsb = const_pool.tile([1, VL], f32, tag="sgvsb")
            nc.gpsimd.dma_start(sgvsb[:], sgv[:])
            b3sb = const_pool.tile([1, VL], f32, tag="b3sb")
            nc.gpsimd.dma_start(b3sb[:], b3[:])

            def emit_l3(v, acc):
                # partition sum on the otherwise-idle Pool engine (frees the
                # PE of 2 matmuls and the ACT of its bias op per variable;
                # PE instruction issue rate is the global binder), then
                # sigma*x + b3 also on Pool, then DMA out
                red = red_pool.tile([128, B], f32, tag="red", name="red")
                m3sb = m3_pool.tile([1, B], f32, tag="m3sb", name="m3sb")
                nc.gpsimd.partition_all_reduce(
                    red[:], acc[:], 128, bass_isa.ReduceOp.add
                )
                nc.gpsimd.tensor_scalar(
                    m3sb[0:1, :],
                    red[0:1, :],
                    sgvsb[0:1, v : v + 1],
                    b3sb[0:1, v : v + 1],
                    op0=mult,
                    op1=add,
                )
                nc.sync.dma_start(out[v : v + 1, :], m3sb[:])

            rep_ctx = tc.For_i(0, reps, 1) if reps > 1 else contextlib.nullcontext()
            with rep_ctx:
                pending = []
                for v in range(VL):
                    w1t = [
                        w1_pool.tile([128, D], mdt, tag="w1t", name=f"w1t_{k}")
                        for k in range(2)
                    ]
                    for kk in range(2):
                        nc.sync.dma_start(
                            w1t[kk][:], w1[v, kk * 128 : (kk + 1) * 128, :]
                        )
                    w2t = [
                        w2_pool.tile([128, D], mdt, tag="w2t", name=f"w2t_{k}")
                        for k in range(4)
                    ]
                    for dd in range(4):
                        nc.sync.dma_start(
                            w2t[dd][:], w2[v, dd * 128 : (dd + 1) * 128, :]
                        )

                    # L1: stationary w1t[kk][:,ms] loaded once, both batch
                    # halves run against it before switching; bias+relu on DVE
                    h1t = [
                        h1_pool.tile([128, B], mdt, tag="h1t", name=f"h1t_{k}")
                        for k in range(4)
                    ]
                    for dd in range(4):
                        ms = slice(dd * 128, (dd + 1) * 128)
                        ps = [
                            ps_pool.tile([128, 512], f32, tag="ps", name="ps")
                            for _ in range(2)
                        ]
                        for kk, xt in ((0, xt0), (1, xt1)):
                            for bb in range(2):
                                bs = slice(bb * 512, (bb + 1) * 512)
                                nc.tensor.matmul(
                                    ps[bb][:],
                                    w1t[kk][:, ms],
                                    xt[:, bs],
                                    start=(kk == 0),
                                    stop=(kk == 1),
                                )
                        # drain the two PSUM halves on DVE and ACT
                        # concurrently: h1t[dd] completes ~2x sooner, so
                        # L2's first accumulation chain (which needs
                        # h1t[3]) never stalls the PE
                        nc.vector.tensor_scalar(
                            h1t[dd][:, 0:512],
                            ps[0][:],
                            b1sb[:, v * 4 + dd : v * 4 + dd + 1],
                            0.0,
                            op0=add,
                            op1=amax,
                        )
                        nc.scalar.activation(
                            h1t[dd][:, 512:B],
                            ps[1][:],
                            Relu,
                            bias=b1sb[:, v * 4 + dd : v * 4 + dd + 1],
                        )

                    # emit the previous variable's L3 here: its tree finishes
                    # while this variable's L1 matmuls run, so the in-order
                    # PE never waits on it
                    if pending:
                        emit_l3(*pending.pop())

                    # L2 + bias+relu on ACT (|W3| rides w2t)
                    tt = [
                        t_pool.tile([128, B], mdt, tag="tt", name=f"tt_{k}")
                        for k in range(4)
                    ]
                    for ee in range(4):
                        ms = slice(ee * 128, (ee + 1) * 128)
                        col = slice(v * 4 + ee, v * 4 + ee + 1)
                        ps2 = [
                            ps_pool.tile([128, 512], f32, tag="ps", name="ps")
                            for _ in range(2)
                        ]
                        for dd in range(4):
                            for bb in range(2):
                                bs = slice(bb * 512, (bb + 1) * 512)
                                nc.tensor.matmul(
                                    ps2[bb][:],
                                    w2t[dd][:, ms],
                                    h1t[dd][:, bs],
                                    start=(dd == 0),
                                    stop=(dd == 3),
                                )
                        for bb in range(2):
                            bs = slice(bb * 512, (bb + 1) * 512)
                            nc.scalar.activation(
                                tt[ee][:, bs],
                                ps2[bb][:],
                                Relu,
                                bias=biAsb[:, col],
                            )

                    # signed tile tree on DVE: the mixed tile 1 applies its
                    # per-partition +-1 vector in one early STT op (t0/t1
                    # are ready first), then two sign-pure subtracts
                    acc = acc_pool.tile([128, B], mdt, tag="acc", name="acc")
                    nc.vector.scalar_tensor_tensor(
                        acc[:],
                        tt[1][:],
                        sg3sb[:, v : v + 1],
                        tt[0][:],
                        op0=mult,
                        op1=add,
                    )
                    nc.vector.tensor_tensor(acc[:], acc[:], tt[2][:], sub)
                    nc.vector.tensor_tensor(acc[:], acc[:], tt[3][:], sub)

                    pending.append((v, acc))
                emit_l3(*pending.pop())

    nc.compile()
    _CACHE[key] = nc
    return nc


def _prep_inputs(x, adjacency, W1, b1, W2, b2, W3, b3):
    mmnp = _np_mm_dtype()
    x = np.asarray(x, np.float32)
    A = np.asarray(adjacency, np.float32)
    W1 = np.asarray(W1, np.float32)
    W2 = np.asarray(W2, np.float32)
    W3 = np.asarray(W3, np.float32)
    b1 = np.asarray(b1, np.float32)
    b2 = np.asarray(b2, np.float32)
    b3 = np.asarray(b3, np.float32)

    W1eff = W1[:, :, :V] + W1[:, :, V : V + 1] * A[:, None, :]
    W1effT = np.ascontiguousarray(W1eff.transpose(0, 2, 1)).astype(mmnp)
    xT = np.ascontiguousarray(x.T).astype(mmnp)

    in_maps = []
    for c in range(NCORES):
        s = slice(c * VL, (c + 1) * VL)
        b1t = np.ascontiguousarray(
            b1[s].reshape(VL, 4, 128).transpose(2, 0, 1).reshape(128, VL * 4)
        )
        w2tc = np.empty((VL, D, D), mmnp)
        biA = np.empty((128, VL * 4), np.float32)
        sg3 = np.empty((128, VL), np.float32)
        onesc = np.empty((1, VL), np.float32)
        for j, gv in enumerate(range(c * VL, (c + 1) * VL)):
            w3v = W3[gv]
            pos = np.flatnonzero(w3v >= 0)
            neg = np.flatnonzero(w3v < 0)
            if len(pos) < 256:
                sigma, plus, minus = 1.0, pos, neg
            else:
                sigma, plus, minus = -1.0, neg, pos
            L = len(plus)
            assert 128 <= L <= 256, f"degenerate sign split L={L}"
            r = L - 128
            # tile0 pure plus; tile1 mixed (plus rows < r, minus rows >= r);
            # tiles 2/3 pure minus
            perm = np.concatenate(
                [
                    plus[0:128],
                    plus[128:L],
                    minus[0 : 128 - r],
                    minus[128 - r : 384 - r],
                ]
            )
            assert len(perm) == D
            aw3 = np.abs(w3v[perm])
            # |W3| rides the matmul weights: scale W2's (permuted) e-rows
            w2tc[j] = np.ascontiguousarray(
                (W2[gv][perm, :] * aw3[:, None]).T
            ).astype(mmnp)
            ab2 = aw3 * b2[gv][perm]
            for ee in range(4):
                biA[:, j * 4 + ee] = ab2[ee * 128 : (ee + 1) * 128]
            sg3[:r, j] = 1.0
            sg3[r:, j] = -1.0
            onesc[0, j] = sigma
        in_maps.append(
            {
                "xT": xT,
                "w1t": np.ascontiguousarray(W1effT[s]),
                "w2t": w2tc,
                "biA": biA,
                "sg3": sg3,
                "b1t": b1t,
                "b3t": np.ascontiguousarray(b3[s].reshape(1, VL)),
                "sgv": np.ascontiguousarray(onesc),
            }
        )
    return in_maps


def kernel(x, adjacency, W1, b1, W2, b2, W3, b3, _trace=False):
    import sys

    if "/opt/trn_rl_repo" not in sys.path:
        sys.path.insert(0, "/opt/trn_rl_repo")
    from concourse.bass_utils import run_bass_kernel_spmd

    nc = _build()
    in_maps = _prep_inputs(x, adjacency, W1, b1, W2, b2, W3, b3)
    res = run_bass_kernel_spmd(
        nc, in_maps, core_ids=list(range(NCORES)), trace=_trace
    )
    kernel.last_results = res
    outT = np.concatenate([res.results[c]["out"] for c in range(NCORES)], axis=0)
    return np.ascontiguousarray(outT.T.astype(np.float32))


kernel.last_results = None
